# revision 15
# baseline (speedup 1.0000x reference)
"""BAP classifier (attention-pooling + linear head) on 8 TRN2 NeuronCores.

Pipeline (reference math):
    A    = sigmoid(einsum('bchw,mc->bmhw', x, Wa) + ba)     # attention maps
    bap  = einsum('bchw,bmhw->bmc', x, A) / (H*W)           # attn-weighted pool
    out  = bap.reshape(B, M*C) @ Wc.T + bc                  # linear head

Sharding:
  Phase 1 — data-parallel over batch (8 batches/core): each core computes
    raw feats rows [8, M*C] (un-normalized bap, transposed per batch on chip).
  Phase 2 — Wc column-parallel (8192 columns of the M*C dim per core): each
    core computes a partial [B, NCLS] logit; host sums partials, applies the
    1/(H*W) scale and bias.

Compute dtype is bf16 on the TensorEngine with fp32 PSUM accumulation.

Perf notes (vs the first working version):
  * Both einsums are column-tiled across the PE array: 4 batches run
    concurrently in 128x32 tiles (col groups), quadrupling PE throughput
    for the M=32-wide matmuls.
  * All bulk DMA is split evenly between the two HWDGE queues (sync +
    scalar) in equal-size "waves" so both queues stream in lockstep;
    the hw-tail (68-partition) x^T tiles go on the gpsimd SWDGE queue as
    one 272-descriptor transfer per quad (68-descriptor transfers only
    fan out to 4 of the 16 SDMA engines).
  * Loads are ordered so compute chases the stream: x (einsum1) for a
    quad, then its x^T c-lo/c-hi halves; einsum2 consumes them nt-chunk
    by nt-chunk, and the final store covers only the last 1024 columns.
  * Phase 2 interleaves ft (stationary) chunks with the first Wc chunks
    so matmuls start ~8us in instead of waiting for the whole ft.
"""
import sys

if "/opt/trn_rl_repo" not in sys.path:
    sys.path.insert(0, "/opt/trn_rl_repo")

import numpy as np

import concourse.bacc as bacc
import concourse.mybir as mybir
from concourse.tile import TileContext
from concourse.bass_utils import run_bass_kernel_spmd
from concourse.masks import make_identity

B, C, H, W = 64, 2048, 14, 14
HW = H * W                     # 196
M, NCLS = 32, 396
NCORES = 8
BPC = B // NCORES              # 8 batches per core
CT = C // 128                  # 16 c-chunks
KTOT = M * C                   # 65536
KPC = KTOT // NCORES           # 8192 Wc columns per core
KT = KPC // 128                # 64 k-tiles per core in phase 2

F32 = mybir.dt.float32
BF16 = mybir.dt.bfloat16

# Run options (test harness may flip these; defaults are what grading uses).
TRACE = False
TRACE_INFO = {}
TRACE_RES = {}

_cache = {}


def _nc():
    return bacc.Bacc(
        "TRN2", target_bir_lowering=False, debug=False, num_devices=NCORES
    )


def _build_phase1():
    """Per-core: x_shard (bf16) [BPC, C, HW] -> raw feats [BPC, M*C].

    c is loaded with the permuted mapping c = p*CT + t (p = partition,
    t = chunk) so every natural-load descriptor is one contiguous run;
    wat arrives host-permuted to the same mapping.  x^T for the BAP einsum
    is supplied pre-transposed by the host ([BPC, HW, C]).

    Both einsums column-tile the PE: batch j of a quad owns col group j
    (PSUM partitions 32j:32j+32), so 4 matmuls run concurrently.
    """
    nc = _nc()
    x = nc.dram_tensor("x", [BPC, C, HW], BF16, kind="ExternalInput")
    xt = nc.dram_tensor("xt", [BPC, HW, C], BF16, kind="ExternalInput")
    wat = nc.dram_tensor("wat", [128, CT, M], BF16, kind="ExternalInput")
    ba2 = nc.dram_tensor("ba2", [64, 1], F32, kind="ExternalInput")
    feats = nc.dram_tensor("feats", [BPC, M * C], BF16, kind="ExternalOutput")

    NP = BPC // 2  # batch pairs per core

    with TileContext(nc) as tc:
        with (
            tc.tile_pool(name="const", bufs=1) as const,
            tc.tile_pool(name="xpool", bufs=8) as xpool,
            tc.tile_pool(name="xtapool", bufs=4) as xtapool,
            tc.tile_pool(name="xtbpool", bufs=2) as xtbpool,
            tc.tile_pool(name="apool", bufs=2) as apool,
            tc.tile_pool(name="atapool", bufs=2) as atapool,
            tc.tile_pool(name="atbpool", bufs=2) as atbpool,
            tc.tile_pool(name="fpool", bufs=2) as fpool,
            tc.tile_pool(name="ps_att", bufs=2, space="PSUM") as ps_att,
            tc.tile_pool(name="ps_tr", bufs=1, space="PSUM") as ps_tr,
            tc.tile_pool(name="ps_bap", bufs=4, space="PSUM") as ps_bap,
        ):
            # --- small constants first on each queue ---
            ba_sb = const.tile([64, 1], F32)
            nc.sync.dma_start(out=ba_sb, in_=ba2.ap())
            wat_sb = const.tile([128, CT, M], BF16)
            nc.scalar.dma_start(out=wat_sb, in_=wat.ap())
            ident = const.tile([128, 128], BF16)
            make_identity(nc, ident)

            # --- bulk loads: one whole-pair dma_start per tensor (1.0-1.6MB
            # each) so the 8 HWDGE semaphore lanes keep ~10MB in flight and
            # the queues never run dry; pairs alternate sync/scalar so the
            # two queues stream concurrently.  The 68-partition hw-tail rows
            # go on the gpsimd SWDGE queue as one 272-descriptor transfer
            # per 4 batches (68-descriptor transfers only reach 4 of 16
            # SDMA engines).
            xp = [None] * NP
            xta = [None] * NP
            xtb = [None] * (NP // 2)
            for q in range(NP // 2):
                xtb[q] = xtbpool.tile([68, 4, C], BF16, tag="xtb", name=f"xtb{q}")
                nc.gpsimd.dma_start(
                    out=xtb[q],
                    in_=xt.ap()[4 * q:4 * q + 4, 128:196, :].rearrange(
                        "b p c -> p b c"
                    ),
                )
            for pr in range(NP):
                b0 = 2 * pr
                eng = nc.sync if pr % 2 == 0 else nc.scalar
                xp[pr] = xpool.tile(
                    [128, 2, CT, HW], BF16, tag="xp", name=f"xp{pr}"
                )
                eng.dma_start(
                    out=xp[pr],
                    in_=x.ap()[b0:b0 + 2].rearrange(
                        "b (p t) f -> p b t f", t=CT
                    ),
                )
                xta[pr] = xtapool.tile(
                    [128, 2, C], BF16, tag="xta", name=f"xta{pr}"
                )
                if pr < 2:
                    eng.dma_start(
                        out=xta[pr],
                        in_=xt.ap()[b0:b0 + 2, 0:128, :].rearrange(
                            "b p c -> p b c"
                        ),
                    )
                else:
                    # last pairs: split x^T by c-halves across both queues so
                    # the queues stay balanced, finish together, and the
                    # compute tail (c-hi chunks of the last pair) is minimal
                    for ch in range(2):
                        c0, c1 = 1024 * ch, 1024 * ch + 1024
                        e2 = eng if ch == 0 else (
                            nc.scalar if eng is nc.sync else nc.sync
                        )
                        e2.dma_start(
                            out=xta[pr][:, :, c0:c1],
                            in_=xt.ap()[b0:b0 + 2, 0:128, c0:c1].rearrange(
                                "b p c -> p b c"
                            ),
                        )

            # --- compute, one batch pair at a time; batch j of the pair owns
            # PE col group j (PSUM partitions 32j:32j+32), so the two
            # batches' matmuls run concurrently in the array.  (Col group 3
            # is a broken hw quadrant, so 2-way is the safe max with the
            # feats layout.) ---
            for pr in range(NP):
                att_ps = ps_att.tile([64, HW], F32, tag="att", name=f"att{pr}")
                for ct in range(CT):
                    for j in range(2):
                        nc.tensor.matmul(
                            att_ps[32 * j:32 * j + 32, :],
                            lhsT=wat_sb[:, ct, :],
                            rhs=xp[pr][:, j, ct, :],
                            start=(ct == 0),
                            stop=(ct == CT - 1),
                            tile_position=(0, 32 * j),
                        )
                a_sb = apool.tile([64, HW], BF16, tag="a_sb", name=f"a_sb{pr}")
                nc.scalar.activation(
                    out=a_sb,
                    in_=att_ps,
                    func=mybir.ActivationFunctionType.Sigmoid,
                    bias=ba_sb,
                )

                # A^T per batch via PE row-tile transposes
                ata = []
                atb = []
                for j in range(2):
                    sl = slice(32 * j, 32 * j + 32)
                    ata_ps = ps_tr.tile([128, M], BF16, tag="ata")
                    nc.tensor.transpose(
                        ata_ps, a_sb[sl, 0:128], ident[sl, sl],
                        tile_position=(32 * j, 0),
                    )
                    ata_j = atapool.tile([128, M], BF16, tag="ata_sb")
                    nc.vector.tensor_copy(out=ata_j, in_=ata_ps)
                    ata.append(ata_j)

                    atb_ps = ps_tr.tile([68, M], BF16, tag="atb")
                    nc.tensor.transpose(
                        atb_ps, a_sb[sl, 128:196], ident[sl, sl],
                        tile_position=(32 * j, 0),
                    )
                    atb_j = atbpool.tile([68, M], BF16, tag="atb_sb")
                    nc.vector.tensor_copy(out=atb_j, in_=atb_ps)
                    atb.append(atb_j)

                # einsum2: bapT[32j+m, c] = sum_hw A[m,hw] x[c,hw]; nt chunks
                # in c-lo->c-hi order so the tail only waits for the last
                # xta half.
                featsq = fpool.tile([64, C], BF16, tag="featsq", name=f"fq{pr}")
                for nt in range(4):
                    bap_ps = ps_bap.tile(
                        [64, 512], F32, tag="bap", name=f"bap{pr}_{nt}"
                    )
                    csl = slice(512 * nt, 512 * nt + 512)
                    for j in range(2):
                        nc.tensor.matmul(
                            bap_ps[32 * j:32 * j + 32, :],
                            lhsT=ata[j],
                            rhs=xta[pr][:, j, csl],
                            start=True,
                            stop=False,
                            tile_position=(0, 32 * j),
                        )
                    for j in range(2):
                        nc.tensor.matmul(
                            bap_ps[32 * j:32 * j + 32, :],
                            lhsT=atb[j],
                            rhs=xtb[pr // 2][:, 2 * (pr % 2) + j, csl],
                            start=False,
                            stop=True,
                            tile_position=(0, 32 * j),
                        )
                    if nt % 2 == 0:
                        nc.vector.tensor_copy(out=featsq[:, csl], in_=bap_ps)
                    else:
                        nc.scalar.copy(out=featsq[:, csl], in_=bap_ps)

                # store: early pairs on the idle SWDGE queue; the last pair
                # in column halves on sync (its loads are done by then)
                fap = feats.ap()[2 * pr:2 * pr + 2].rearrange(
                    "b (m c) -> (b m) c", m=M
                )
                if pr < NP - 1:
                    nc.gpsimd.dma_start(out=fap, in_=featsq)
                else:
                    nc.sync.dma_start(out=fap[:, 0:1024], in_=featsq[:, 0:1024])
                    nc.sync.dma_start(
                        out=fap[:, 1024:2048], in_=featsq[:, 1024:2048]
                    )
    nc.compile()
    return nc


def _build_phase2():
    """Per-core: featsT slice (partition-major, bf16) x WcT slice (bf16)
    -> partial [B, NCLS] (fp32)."""
    nc = _nc()
    ft = nc.dram_tensor("ft", [128, KT, B], BF16, kind="ExternalInput")
    wct = nc.dram_tensor("wct", [128, KT, NCLS], BF16, kind="ExternalInput")
    part = nc.dram_tensor("part", [B, NCLS], F32, kind="ExternalOutput")

    # (kt0, ch, engine_idx): 0 = sync, 1 = scalar.  A tiny first chunk gets
    # matmuls started early; later chunks are big (0.8-1.6MB) so the 8 HWDGE
    # semaphore lanes keep enough bytes in flight to never starve the queues.
    wchunks = [
        (0, 4, 0), (4, 4, 1), (8, 8, 0), (16, 8, 1), (24, 16, 0),
        (40, 16, 1), (56, 4, 0), (60, 4, 1),
    ]

    with TileContext(nc) as tc:
        with (
            tc.tile_pool(name="fpool", bufs=1) as fpool,
            tc.tile_pool(name="wpool", bufs=8) as wpool,
            tc.tile_pool(name="opool", bufs=1) as opool,
            tc.tile_pool(name="ps_out", bufs=1, space="PSUM") as ps_out,
        ):
            engs = [nc.sync, nc.scalar]
            ft_sb = fpool.tile([128, KT, B], BF16)
            # emission order per engine == queue order: small ft chunk and
            # first wct chunk first, so kt=0 matmuls start ~10us in
            nc.sync.dma_start(out=ft_sb[:, 0:8, :], in_=ft.ap()[:, 0:8, :])
            nc.scalar.dma_start(out=ft_sb[:, 8:32, :], in_=ft.ap()[:, 8:32, :])
            w_sb = {}
            for kt0, ch, e in wchunks[:2]:
                w_sb[kt0] = wpool.tile([128, ch, NCLS], BF16, tag="w",
                                       name=f"w{kt0}")
                engs[e].dma_start(
                    out=w_sb[kt0], in_=wct.ap()[:, kt0:kt0 + ch, :]
                )
            nc.sync.dma_start(out=ft_sb[:, 32:48, :], in_=ft.ap()[:, 32:48, :])
            nc.scalar.dma_start(out=ft_sb[:, 48:64, :], in_=ft.ap()[:, 48:64, :])
            for kt0, ch, e in wchunks[2:]:
                w_sb[kt0] = wpool.tile([128, ch, NCLS], BF16, tag="w",
                                       name=f"w{kt0}")
                engs[e].dma_start(
                    out=w_sb[kt0], in_=wct.ap()[:, kt0:kt0 + ch, :]
                )

            # col-paired matmuls: even kt accumulate into PSUM partitions
            # 0:64 (col group pair 0), odd kt into 64:128, so consecutive
            # k-tiles run concurrently in the PE array.
            out_ps = ps_out.tile([128, NCLS], F32)
            for kt0, ch, e in wchunks:
                for kl in range(ch):
                    kt = kt0 + kl
                    half = kt % 2
                    nc.tensor.matmul(
                        out_ps[64 * half:64 * half + 64, :],
                        lhsT=ft_sb[:, kt, :],
                        rhs=w_sb[kt0][:, kl, :],
                        start=(kt < 2),
                        stop=(kt >= KT - 2),
                        tile_position=(0, 64 * half),
                    )
            hi_sb = opool.tile([B, NCLS], F32, tag="hi")
            nc.scalar.copy(out=hi_sb, in_=out_ps[64:128, :])
            out_sb = opool.tile([B, NCLS], F32, tag="out")
            nc.vector.tensor_tensor(
                out=out_sb, in0=out_ps[0:64, :], in1=hi_sb,
                op=mybir.AluOpType.add,
            )
            nc.sync.dma_start(out=part.ap(), in_=out_sb)
    nc.compile()
    return nc


def _install_ntff_hook():
    import types

    import trn_agent_boot.trn_boot as tb
    import concourse.bass_utils as bu

    hook = tb._ntff_profile_via_ctypes("/opt/axon/libaxon_pjrt.so")
    mod = types.ModuleType("antenv.axon_hooks")
    mod.get_axon_ntff_profile_hook = lambda: hook
    sys.modules["antenv.axon_hooks"] = mod
    bu.upload_artifacts = lambda tmpdir: "(skipped)"


def _run(nc, in_maps, label):
    core_ids = list(range(NCORES))
    if TRACE:
        _install_ntff_hook()
        res = run_bass_kernel_spmd(nc, in_maps, core_ids, trace=True)
        TRACE_INFO[label] = res.exec_time_ns
        TRACE_RES[label] = res
    else:
        res = run_bass_kernel_spmd(nc, in_maps, core_ids)
    return res.results


def kernel(x, Wa, ba, Wc, bc):
    import ml_dtypes

    bf16 = np.dtype(ml_dtypes.bfloat16)
    x3 = np.ascontiguousarray(x, dtype=np.float32).reshape(B, C, HW)
    x = x3.astype(bf16)
    xt = np.ascontiguousarray(x.transpose(0, 2, 1))  # [B, HW, C] bf16
    # wat[p, t, m] = Wa[m, p*CT + t] — matches the kernel's permuted c layout
    wat = np.ascontiguousarray(Wa.T, dtype=np.float32).astype(bf16).reshape(
        128, CT, M
    )
    ba2 = np.tile(
        np.ascontiguousarray(ba, dtype=np.float32).reshape(M, 1), (2, 1)
    )
    wct = np.ascontiguousarray(Wc.T, dtype=np.float32).astype(bf16)  # [KTOT, NCLS]

    if "p1" not in _cache:
        _cache["p1"] = _build_phase1()
    if "p2" not in _cache:
        _cache["p2"] = _build_phase2()

    in1 = [
        {
            "x": x[i * BPC: (i + 1) * BPC],
            "xt": xt[i * BPC: (i + 1) * BPC],
            "wat": wat,
            "ba2": ba2,
        }
        for i in range(NCORES)
    ]
    res1 = _run(_cache["p1"], in1, "phase1")
    feats = np.concatenate([r["feats"] for r in res1], axis=0)  # [B, KTOT] bf16

    # ft[p, t, b] = feats[b, kslice + t*128 + p] (partition-major, bf16)
    featsT = np.ascontiguousarray(feats.T)  # [KTOT, B]
    in2 = [
        {
            "ft": np.ascontiguousarray(
                featsT[i * KPC: (i + 1) * KPC].reshape(KT, 128, B).transpose(
                    1, 0, 2
                )
            ),
            "wct": np.ascontiguousarray(
                wct[i * KPC: (i + 1) * KPC].reshape(KT, 128, NCLS).transpose(
                    1, 0, 2
                )
            ),
        }
        for i in range(NCORES)
    ]
    res2 = _run(_cache["p2"], in2, "phase2")
    parts = np.stack([r["part"] for r in res2], axis=0)  # [NCORES, B, NCLS]

    logits = parts.sum(axis=0) / float(HW) + np.asarray(bc, dtype=np.float32)
    return logits.astype(np.float32)


# revision 20
# speedup vs baseline: 1.2237x; 1.2237x over previous
"""BAP classifier (attention-pooling + linear head) on 8 TRN2 NeuronCores.

Pipeline (reference math):
    A    = sigmoid(einsum('bchw,mc->bmhw', x, Wa) + ba)     # attention maps
    bap  = einsum('bchw,bmhw->bmc', x, A) / (H*W)           # attn-weighted pool
    out  = bap.reshape(B, M*C) @ Wc.T + bc                  # linear head

Sharding:
  Phase 1 — data-parallel over batch (8 batches/core): each core computes
    raw feats rows [8, M*C] (un-normalized bap, transposed per batch on chip).
  Phase 2 — Wc column-parallel (8192 columns of the M*C dim per core): each
    core computes a partial [B, NCLS] logit; host sums partials, applies the
    1/(H*W) scale and bias.

Compute dtype is bf16 on the TensorEngine with fp32 PSUM accumulation.

Perf notes (vs the first working version):
  * Both einsums are column-tiled across the PE array: 4 batches run
    concurrently in 128x32 tiles (col groups), quadrupling PE throughput
    for the M=32-wide matmuls.
  * All bulk DMA is split evenly between the two HWDGE queues (sync +
    scalar) in equal-size "waves" so both queues stream in lockstep;
    the hw-tail (68-partition) x^T tiles go on the gpsimd SWDGE queue as
    one 272-descriptor transfer per quad (68-descriptor transfers only
    fan out to 4 of the 16 SDMA engines).
  * Loads are ordered so compute chases the stream: x (einsum1) for a
    quad, then its x^T c-lo/c-hi halves; einsum2 consumes them nt-chunk
    by nt-chunk, and the final store covers only the last 1024 columns.
  * Phase 2 interleaves ft (stationary) chunks with the first Wc chunks
    so matmuls start ~8us in instead of waiting for the whole ft.
"""
import sys

if "/opt/trn_rl_repo" not in sys.path:
    sys.path.insert(0, "/opt/trn_rl_repo")

import numpy as np

import concourse.bacc as bacc
import concourse.mybir as mybir
from concourse.tile import TileContext
from concourse.bass_utils import run_bass_kernel_spmd
from concourse.masks import make_identity

B, C, H, W = 64, 2048, 14, 14
HW = H * W                     # 196
M, NCLS = 32, 396
NCORES = 8
BPC = B // NCORES              # 8 batches per core
CT = C // 128                  # 16 c-chunks
KTOT = M * C                   # 65536
KPC = KTOT // NCORES           # 8192 Wc columns per core
KT = KPC // 128                # 64 k-tiles per core in phase 2

F32 = mybir.dt.float32
BF16 = mybir.dt.bfloat16

# Run options (test harness may flip these; defaults are what grading uses).
TRACE = False
TRACE_INFO = {}
TRACE_RES = {}

_cache = {}


def _nc():
    return bacc.Bacc(
        "TRN2", target_bir_lowering=False, debug=False, num_devices=NCORES
    )


def _build_phase1():
    """Per-core: x_shard (bf16) [BPC, C, HW] -> raw feats [BPC, M*C].

    c is loaded with the permuted mapping c = p*CT + t (p = partition,
    t = chunk) so every natural-load descriptor is one contiguous run;
    wat arrives host-permuted to the same mapping.  x^T for the BAP einsum
    is supplied pre-transposed by the host ([BPC, HW, C]).

    Both einsums column-tile the PE: batch j of a quad owns col group j
    (PSUM partitions 32j:32j+32), so 4 matmuls run concurrently.
    """
    nc = _nc()
    x = nc.dram_tensor("x", [BPC, C, HW], BF16, kind="ExternalInput")
    xt = nc.dram_tensor("xt", [BPC, HW, C], BF16, kind="ExternalInput")
    wat = nc.dram_tensor("wat", [128, CT, M], BF16, kind="ExternalInput")
    ba2 = nc.dram_tensor("ba2", [64, 1], F32, kind="ExternalInput")
    feats = nc.dram_tensor("feats", [BPC, M * C], BF16, kind="ExternalOutput")

    NP = BPC // 2  # batch pairs per core

    with TileContext(nc) as tc:
        with (
            tc.tile_pool(name="const", bufs=1) as const,
            tc.tile_pool(name="xpool", bufs=8) as xpool,
            tc.tile_pool(name="xtapool", bufs=4) as xtapool,
            tc.tile_pool(name="xtbpool", bufs=4) as xtbpool,
            tc.tile_pool(name="apool", bufs=2) as apool,
            tc.tile_pool(name="atapool", bufs=2) as atapool,
            tc.tile_pool(name="atbpool", bufs=2) as atbpool,
            tc.tile_pool(name="fpool", bufs=2) as fpool,
            tc.tile_pool(name="ps_att", bufs=2, space="PSUM") as ps_att,
            tc.tile_pool(name="ps_tr", bufs=1, space="PSUM") as ps_tr,
            tc.tile_pool(name="ps_bap", bufs=4, space="PSUM") as ps_bap,
        ):
            # --- small constants first on each queue ---
            ba_sb = const.tile([64, 1], F32)
            nc.sync.dma_start(out=ba_sb, in_=ba2.ap())
            wat_sb = const.tile([128, CT, M], BF16)
            nc.scalar.dma_start(out=wat_sb, in_=wat.ap())
            ident = const.tile([128, 128], BF16)
            make_identity(nc, ident)

            # --- bulk loads: one whole-pair dma_start per tensor (1.0-1.6MB
            # each) so the 8 HWDGE semaphore lanes keep ~10MB in flight and
            # the queues never run dry; pairs alternate sync/scalar so the
            # two queues stream concurrently.
            #
            # Every transfer spans exactly 128 partitions: partition counts
            # below 128 fan out to only a few of the 16 SDMA engines (a
            # 68-partition transfer uses 4) and wreck throughput.  So the
            # hw dim is covered as rows 0:128 (xta) plus rows 68:196 (xtb),
            # and the einsum contracts hw 0:68 from xta and 68:196 from xtb.
            xp = [None] * NP
            xta = [None] * NP
            xtb = [None] * NP
            for pr in range(NP):
                b0 = 2 * pr
                eng = nc.sync if pr % 2 == 0 else nc.scalar
                oth = nc.scalar if pr % 2 == 0 else nc.sync
                xp[pr] = xpool.tile(
                    [128, 2, CT, HW], BF16, tag="xp", name=f"xp{pr}"
                )
                eng.dma_start(
                    out=xp[pr],
                    in_=x.ap()[b0:b0 + 2].rearrange(
                        "b (p t) f -> p b t f", t=CT
                    ),
                )
                xta[pr] = xtapool.tile(
                    [128, 2, C], BF16, tag="xta", name=f"xta{pr}"
                )
                xtb[pr] = xtbpool.tile(
                    [128, 2, C], BF16, tag="xtb", name=f"xtb{pr}"
                )
                if pr < NP - 1:
                    eng.dma_start(
                        out=xta[pr],
                        in_=xt.ap()[b0:b0 + 2, 0:128, :].rearrange(
                            "b p c -> p b c"
                        ),
                    )
                    oth.dma_start(
                        out=xtb[pr],
                        in_=xt.ap()[b0:b0 + 2, 68:196, :].rearrange(
                            "b p c -> p b c"
                        ),
                    )
                else:
                    # last pair: split x^T by c-halves, xta on one queue and
                    # xtb on the other, so both queues finish together and
                    # the compute tail (c-hi chunks) is minimal
                    for ch in range(2):
                        c0, c1 = 1024 * ch, 1024 * ch + 1024
                        eng.dma_start(
                            out=xta[pr][:, :, c0:c1],
                            in_=xt.ap()[b0:b0 + 2, 0:128, c0:c1].rearrange(
                                "b p c -> p b c"
                            ),
                        )
                        oth.dma_start(
                            out=xtb[pr][:, :, c0:c1],
                            in_=xt.ap()[b0:b0 + 2, 68:196, c0:c1].rearrange(
                                "b p c -> p b c"
                            ),
                        )

            # --- compute, one batch pair at a time; batch j of the pair owns
            # PE col group j (PSUM partitions 32j:32j+32), so the two
            # batches' matmuls run concurrently in the array.  (Col group 3
            # is a broken hw quadrant, so 2-way is the safe max with the
            # feats layout.) ---
            for pr in range(NP):
                att_ps = ps_att.tile([64, HW], F32, tag="att", name=f"att{pr}")
                for ct in range(CT):
                    for j in range(2):
                        nc.tensor.matmul(
                            att_ps[32 * j:32 * j + 32, :],
                            lhsT=wat_sb[:, ct, :],
                            rhs=xp[pr][:, j, ct, :],
                            start=(ct == 0),
                            stop=(ct == CT - 1),
                            tile_position=(0, 32 * j),
                        )
                a_sb = apool.tile([64, HW], BF16, tag="a_sb", name=f"a_sb{pr}")
                nc.scalar.activation(
                    out=a_sb,
                    in_=att_ps,
                    func=mybir.ActivationFunctionType.Sigmoid,
                    bias=ba_sb,
                )

                # A^T per batch via PE row-tile transposes: ata = A^T rows
                # 0:68 (contracted against xta partitions 0:68), atb = A^T
                # rows 68:196 (contracted against the full 128-row xtb tile)
                ata = []
                atb = []
                for j in range(2):
                    sl = slice(32 * j, 32 * j + 32)
                    ata_ps = ps_tr.tile([68, M], BF16, tag="ata")
                    nc.tensor.transpose(
                        ata_ps, a_sb[sl, 0:68], ident[sl, sl],
                        tile_position=(32 * j, 0),
                    )
                    ata_j = atapool.tile([68, M], BF16, tag="ata_sb")
                    nc.vector.tensor_copy(out=ata_j, in_=ata_ps)
                    ata.append(ata_j)

                    atb_ps = ps_tr.tile([128, M], BF16, tag="atb")
                    nc.tensor.transpose(
                        atb_ps, a_sb[sl, 68:196], ident[sl, sl],
                        tile_position=(32 * j, 0),
                    )
                    atb_j = atbpool.tile([128, M], BF16, tag="atb_sb")
                    nc.vector.tensor_copy(out=atb_j, in_=atb_ps)
                    atb.append(atb_j)

                # einsum2: bapT[32j+m, c] = sum_hw A[m,hw] x[c,hw]; nt chunks
                # in c-lo->c-hi order so the tail only waits for the last
                # xta half.
                featsq = fpool.tile([64, C], BF16, tag="featsq", name=f"fq{pr}")
                for nt in range(4):
                    bap_ps = ps_bap.tile(
                        [64, 512], F32, tag="bap", name=f"bap{pr}_{nt}"
                    )
                    csl = slice(512 * nt, 512 * nt + 512)
                    for j in range(2):
                        nc.tensor.matmul(
                            bap_ps[32 * j:32 * j + 32, :],
                            lhsT=ata[j],
                            rhs=xta[pr][0:68, j, csl],
                            start=True,
                            stop=False,
                            tile_position=(0, 32 * j),
                        )
                    for j in range(2):
                        nc.tensor.matmul(
                            bap_ps[32 * j:32 * j + 32, :],
                            lhsT=atb[j],
                            rhs=xtb[pr][:, j, csl],
                            start=False,
                            stop=True,
                            tile_position=(0, 32 * j),
                        )
                    if nt % 2 == 0:
                        nc.vector.tensor_copy(out=featsq[:, csl], in_=bap_ps)
                    else:
                        nc.scalar.copy(out=featsq[:, csl], in_=bap_ps)

                # store: early pairs on the idle SWDGE queue; the last pair
                # in column halves on sync (its loads are done by then)
                fap = feats.ap()[2 * pr:2 * pr + 2].rearrange(
                    "b (m c) -> (b m) c", m=M
                )
                if pr < NP - 1:
                    nc.gpsimd.dma_start(out=fap, in_=featsq)
                else:
                    nc.sync.dma_start(out=fap[:, 0:1024], in_=featsq[:, 0:1024])
                    nc.sync.dma_start(
                        out=fap[:, 1024:2048], in_=featsq[:, 1024:2048]
                    )
    nc.compile()
    return nc


def _build_phase2():
    """Per-core: featsT slice (partition-major, bf16) x WcT slice (bf16)
    -> partial [B, NCLS] (fp32)."""
    nc = _nc()
    ft = nc.dram_tensor("ft", [128, KT, B], BF16, kind="ExternalInput")
    wct = nc.dram_tensor("wct", [128, KT, NCLS], BF16, kind="ExternalInput")
    part = nc.dram_tensor("part", [B, NCLS], F32, kind="ExternalOutput")

    # (kt0, ch, engine_idx): 0 = sync, 1 = scalar.  A tiny first chunk gets
    # matmuls started early; later chunks are big (0.8-1.6MB) so the 8 HWDGE
    # semaphore lanes keep enough bytes in flight to never starve the queues.
    wchunks = [
        (0, 4, 0), (4, 4, 1), (8, 8, 0), (16, 8, 1), (24, 16, 0),
        (40, 16, 1), (56, 4, 0), (60, 4, 1),
    ]

    with TileContext(nc) as tc:
        with (
            tc.tile_pool(name="fpool", bufs=1) as fpool,
            tc.tile_pool(name="wpool", bufs=8) as wpool,
            tc.tile_pool(name="opool", bufs=1) as opool,
            tc.tile_pool(name="ps_out", bufs=1, space="PSUM") as ps_out,
        ):
            engs = [nc.sync, nc.scalar]
            ft_sb = fpool.tile([128, KT, B], BF16)
            # emission order per engine == queue order: small ft chunk and
            # first wct chunk first, so kt=0 matmuls start ~10us in
            nc.sync.dma_start(out=ft_sb[:, 0:8, :], in_=ft.ap()[:, 0:8, :])
            nc.scalar.dma_start(out=ft_sb[:, 8:32, :], in_=ft.ap()[:, 8:32, :])
            w_sb = {}
            for kt0, ch, e in wchunks[:2]:
                w_sb[kt0] = wpool.tile([128, ch, NCLS], BF16, tag="w",
                                       name=f"w{kt0}")
                engs[e].dma_start(
                    out=w_sb[kt0], in_=wct.ap()[:, kt0:kt0 + ch, :]
                )
            nc.sync.dma_start(out=ft_sb[:, 32:48, :], in_=ft.ap()[:, 32:48, :])
            nc.scalar.dma_start(out=ft_sb[:, 48:64, :], in_=ft.ap()[:, 48:64, :])
            for kt0, ch, e in wchunks[2:]:
                w_sb[kt0] = wpool.tile([128, ch, NCLS], BF16, tag="w",
                                       name=f"w{kt0}")
                engs[e].dma_start(
                    out=w_sb[kt0], in_=wct.ap()[:, kt0:kt0 + ch, :]
                )

            # col-paired matmuls: even kt accumulate into PSUM partitions
            # 0:64 (col group pair 0), odd kt into 64:128, so consecutive
            # k-tiles run concurrently in the PE array.
            out_ps = ps_out.tile([128, NCLS], F32)
            for kt0, ch, e in wchunks:
                for kl in range(ch):
                    kt = kt0 + kl
                    half = kt % 2
                    nc.tensor.matmul(
                        out_ps[64 * half:64 * half + 64, :],
                        lhsT=ft_sb[:, kt, :],
                        rhs=w_sb[kt0][:, kl, :],
                        start=(kt < 2),
                        stop=(kt >= KT - 2),
                        tile_position=(0, 64 * half),
                    )
            hi_sb = opool.tile([B, NCLS], F32, tag="hi")
            nc.scalar.copy(out=hi_sb, in_=out_ps[64:128, :])
            out_sb = opool.tile([B, NCLS], F32, tag="out")
            nc.vector.tensor_tensor(
                out=out_sb, in0=out_ps[0:64, :], in1=hi_sb,
                op=mybir.AluOpType.add,
            )
            nc.sync.dma_start(out=part.ap(), in_=out_sb)
    nc.compile()
    return nc


def _install_ntff_hook():
    import types

    import trn_agent_boot.trn_boot as tb
    import concourse.bass_utils as bu

    hook = tb._ntff_profile_via_ctypes("/opt/axon/libaxon_pjrt.so")
    mod = types.ModuleType("antenv.axon_hooks")
    mod.get_axon_ntff_profile_hook = lambda: hook
    sys.modules["antenv.axon_hooks"] = mod
    bu.upload_artifacts = lambda tmpdir: "(skipped)"


def _run(nc, in_maps, label):
    core_ids = list(range(NCORES))
    if TRACE:
        _install_ntff_hook()
        res = run_bass_kernel_spmd(nc, in_maps, core_ids, trace=True)
        TRACE_INFO[label] = res.exec_time_ns
        TRACE_RES[label] = res
    else:
        res = run_bass_kernel_spmd(nc, in_maps, core_ids)
    return res.results


def kernel(x, Wa, ba, Wc, bc):
    import ml_dtypes

    bf16 = np.dtype(ml_dtypes.bfloat16)
    x3 = np.ascontiguousarray(x, dtype=np.float32).reshape(B, C, HW)
    x = x3.astype(bf16)
    xt = np.ascontiguousarray(x.transpose(0, 2, 1))  # [B, HW, C] bf16
    # wat[p, t, m] = Wa[m, p*CT + t] — matches the kernel's permuted c layout
    wat = np.ascontiguousarray(Wa.T, dtype=np.float32).astype(bf16).reshape(
        128, CT, M
    )
    ba2 = np.tile(
        np.ascontiguousarray(ba, dtype=np.float32).reshape(M, 1), (2, 1)
    )
    wct = np.ascontiguousarray(Wc.T, dtype=np.float32).astype(bf16)  # [KTOT, NCLS]

    if "p1" not in _cache:
        _cache["p1"] = _build_phase1()
    if "p2" not in _cache:
        _cache["p2"] = _build_phase2()

    in1 = [
        {
            "x": x[i * BPC: (i + 1) * BPC],
            "xt": xt[i * BPC: (i + 1) * BPC],
            "wat": wat,
            "ba2": ba2,
        }
        for i in range(NCORES)
    ]
    res1 = _run(_cache["p1"], in1, "phase1")
    feats = np.concatenate([r["feats"] for r in res1], axis=0)  # [B, KTOT] bf16

    # ft[p, t, b] = feats[b, kslice + t*128 + p] (partition-major, bf16)
    featsT = np.ascontiguousarray(feats.T)  # [KTOT, B]
    in2 = [
        {
            "ft": np.ascontiguousarray(
                featsT[i * KPC: (i + 1) * KPC].reshape(KT, 128, B).transpose(
                    1, 0, 2
                )
            ),
            "wct": np.ascontiguousarray(
                wct[i * KPC: (i + 1) * KPC].reshape(KT, 128, NCLS).transpose(
                    1, 0, 2
                )
            ),
        }
        for i in range(NCORES)
    ]
    res2 = _run(_cache["p2"], in2, "phase2")
    parts = np.stack([r["part"] for r in res2], axis=0)  # [NCORES, B, NCLS]

    logits = parts.sum(axis=0) / float(HW) + np.asarray(bc, dtype=np.float32)
    return logits.astype(np.float32)


# revision 23
# speedup vs baseline: 1.2499x; 1.0213x over previous
"""BAP classifier (attention-pooling + linear head) on 8 TRN2 NeuronCores.

Pipeline (reference math):
    A    = sigmoid(einsum('bchw,mc->bmhw', x, Wa) + ba)     # attention maps
    bap  = einsum('bchw,bmhw->bmc', x, A) / (H*W)           # attn-weighted pool
    out  = bap.reshape(B, M*C) @ Wc.T + bc                  # linear head

Sharding:
  Phase 1 — data-parallel over batch (8 batches/core): each core computes
    raw feats rows [8, M*C] (un-normalized bap, transposed per batch on chip).
  Phase 2 — Wc column-parallel (8192 columns of the M*C dim per core): each
    core computes a partial [B, NCLS] logit; host sums partials, applies the
    1/(H*W) scale and bias.

Compute dtype is bf16 on the TensorEngine with fp32 PSUM accumulation.

Perf notes (vs the first working version):
  * Both einsums are column-tiled across the PE array: 4 batches run
    concurrently in 128x32 tiles (col groups), quadrupling PE throughput
    for the M=32-wide matmuls.
  * All bulk DMA is split evenly between the two HWDGE queues (sync +
    scalar) in equal-size "waves" so both queues stream in lockstep;
    the hw-tail (68-partition) x^T tiles go on the gpsimd SWDGE queue as
    one 272-descriptor transfer per quad (68-descriptor transfers only
    fan out to 4 of the 16 SDMA engines).
  * Loads are ordered so compute chases the stream: x (einsum1) for a
    quad, then its x^T c-lo/c-hi halves; einsum2 consumes them nt-chunk
    by nt-chunk, and the final store covers only the last 1024 columns.
  * Phase 2 interleaves ft (stationary) chunks with the first Wc chunks
    so matmuls start ~8us in instead of waiting for the whole ft.
"""
import sys

if "/opt/trn_rl_repo" not in sys.path:
    sys.path.insert(0, "/opt/trn_rl_repo")

import numpy as np

import concourse.bacc as bacc
import concourse.mybir as mybir
from concourse.tile import TileContext
from concourse.bass_utils import run_bass_kernel_spmd
from concourse.masks import make_identity

B, C, H, W = 64, 2048, 14, 14
HW = H * W                     # 196
M, NCLS = 32, 396
NCORES = 8
BPC = B // NCORES              # 8 batches per core
CT = C // 128                  # 16 c-chunks
KTOT = M * C                   # 65536
KPC = KTOT // NCORES           # 8192 Wc columns per core
KT = KPC // 128                # 64 k-tiles per core in phase 2

F32 = mybir.dt.float32
BF16 = mybir.dt.bfloat16

# Run options (test harness may flip these; defaults are what grading uses).
TRACE = False
TRACE_INFO = {}
TRACE_RES = {}

_cache = {}


def _nc():
    return bacc.Bacc(
        "TRN2", target_bir_lowering=False, debug=False, num_devices=NCORES
    )


def _build_phase1():
    """Per-core: x_shard (bf16) [BPC, C, HW] -> raw feats [BPC, M*C].

    c is loaded with the permuted mapping c = p*CT + t (p = partition,
    t = chunk) so every natural-load descriptor is one contiguous run;
    wat arrives host-permuted to the same mapping.  x^T for the BAP einsum
    is supplied pre-transposed by the host ([BPC, HW, C]).

    Both einsums column-tile the PE: batch j of a quad owns col group j
    (PSUM partitions 32j:32j+32), so 4 matmuls run concurrently.
    """
    nc = _nc()
    x = nc.dram_tensor("x", [BPC, C, HW], BF16, kind="ExternalInput")
    xt = nc.dram_tensor("xt", [BPC, HW, C], BF16, kind="ExternalInput")
    wat = nc.dram_tensor("wat", [128, CT, M], BF16, kind="ExternalInput")
    ba2 = nc.dram_tensor("ba2", [64, 1], F32, kind="ExternalInput")
    feats = nc.dram_tensor("feats", [BPC, M * C], BF16, kind="ExternalOutput")

    NP = BPC // 2  # batch pairs per core

    with TileContext(nc) as tc:
        with (
            tc.tile_pool(name="const", bufs=1) as const,
            tc.tile_pool(name="xpool", bufs=8) as xpool,
            tc.tile_pool(name="xtapool", bufs=4) as xtapool,
            tc.tile_pool(name="xtbpool", bufs=4) as xtbpool,
            tc.tile_pool(name="apool", bufs=2) as apool,
            tc.tile_pool(name="atapool", bufs=2) as atapool,
            tc.tile_pool(name="atbpool", bufs=2) as atbpool,
            tc.tile_pool(name="fpool", bufs=2) as fpool,
            tc.tile_pool(name="ps_att", bufs=2, space="PSUM") as ps_att,
            tc.tile_pool(name="ps_tr", bufs=1, space="PSUM") as ps_tr,
            tc.tile_pool(name="ps_bap", bufs=4, space="PSUM") as ps_bap,
        ):
            # --- small constants first on each queue ---
            ba_sb = const.tile([64, 1], F32)
            nc.sync.dma_start(out=ba_sb, in_=ba2.ap())
            wat_sb = const.tile([128, CT, M], BF16)
            nc.scalar.dma_start(out=wat_sb, in_=wat.ap())
            ident = const.tile([128, 128], BF16)
            make_identity(nc, ident)

            # --- bulk loads: one whole-pair dma_start per tensor (1.0-1.6MB
            # each) so the 8 HWDGE semaphore lanes keep ~10MB in flight and
            # the queues never run dry; pairs alternate sync/scalar so the
            # two queues stream concurrently.
            #
            # Every transfer spans exactly 128 partitions: partition counts
            # below 128 fan out to only a few of the 16 SDMA engines (a
            # 68-partition transfer uses 4) and wreck throughput.  So the
            # hw dim is covered as rows 0:128 (xta) plus rows 68:196 (xtb),
            # and the einsum contracts hw 0:68 from xta and 68:196 from xtb.
            xp = [None] * NP
            xta = [None] * NP
            xtb = [None] * NP
            for pr in range(NP):
                b0 = 2 * pr
                eng = nc.sync if pr % 2 == 0 else nc.scalar
                oth = nc.scalar if pr % 2 == 0 else nc.sync
                xp[pr] = xpool.tile(
                    [128, 2, CT, HW], BF16, tag="xp", name=f"xp{pr}"
                )
                eng.dma_start(
                    out=xp[pr],
                    in_=x.ap()[b0:b0 + 2].rearrange(
                        "b (p t) f -> p b t f", t=CT
                    ),
                )
                xta[pr] = xtapool.tile(
                    [128, 2, C], BF16, tag="xta", name=f"xta{pr}"
                )
                xtb[pr] = xtbpool.tile(
                    [128, 2, C], BF16, tag="xtb", name=f"xtb{pr}"
                )
                if pr < NP - 1:
                    eng.dma_start(
                        out=xta[pr],
                        in_=xt.ap()[b0:b0 + 2, 0:128, :].rearrange(
                            "b p c -> p b c"
                        ),
                    )
                    oth.dma_start(
                        out=xtb[pr],
                        in_=xt.ap()[b0:b0 + 2, 68:196, :].rearrange(
                            "b p c -> p b c"
                        ),
                    )
                else:
                    # last pair: split x^T by c-halves, xta on one queue and
                    # xtb on the other, so both queues finish together and
                    # the compute tail (c-hi chunks) is minimal
                    for ch in range(2):
                        c0, c1 = 1024 * ch, 1024 * ch + 1024
                        eng.dma_start(
                            out=xta[pr][:, :, c0:c1],
                            in_=xt.ap()[b0:b0 + 2, 0:128, c0:c1].rearrange(
                                "b p c -> p b c"
                            ),
                        )
                        oth.dma_start(
                            out=xtb[pr][:, :, c0:c1],
                            in_=xt.ap()[b0:b0 + 2, 68:196, c0:c1].rearrange(
                                "b p c -> p b c"
                            ),
                        )

            # --- compute, one batch pair at a time; batch j of the pair owns
            # PE col group j (PSUM partitions 32j:32j+32), so the two
            # batches' matmuls run concurrently in the array.  (Col group 3
            # is a broken hw quadrant, so 2-way is the safe max with the
            # feats layout.) ---
            for pr in range(NP):
                att_ps = ps_att.tile([64, HW], F32, tag="att", name=f"att{pr}")
                for ct in range(CT):
                    for j in range(2):
                        nc.tensor.matmul(
                            att_ps[32 * j:32 * j + 32, :],
                            lhsT=wat_sb[:, ct, :],
                            rhs=xp[pr][:, j, ct, :],
                            start=(ct == 0),
                            stop=(ct == CT - 1),
                            tile_position=(0, 32 * j),
                        )
                a_sb = apool.tile([64, HW], BF16, tag="a_sb", name=f"a_sb{pr}")
                nc.scalar.activation(
                    out=a_sb,
                    in_=att_ps,
                    func=mybir.ActivationFunctionType.Sigmoid,
                    bias=ba_sb,
                )

                # A^T per batch via PE row-tile transposes: ata = A^T rows
                # 0:68 (contracted against xta partitions 0:68), atb = A^T
                # rows 68:196 (contracted against the full 128-row xtb tile)
                ata = []
                atb = []
                for j in range(2):
                    sl = slice(32 * j, 32 * j + 32)
                    ata_ps = ps_tr.tile([68, M], BF16, tag="ata")
                    nc.tensor.transpose(
                        ata_ps, a_sb[sl, 0:68], ident[sl, sl],
                        tile_position=(32 * j, 0),
                    )
                    ata_j = atapool.tile([68, M], BF16, tag="ata_sb")
                    nc.vector.tensor_copy(out=ata_j, in_=ata_ps)
                    ata.append(ata_j)

                    atb_ps = ps_tr.tile([128, M], BF16, tag="atb")
                    nc.tensor.transpose(
                        atb_ps, a_sb[sl, 68:196], ident[sl, sl],
                        tile_position=(32 * j, 0),
                    )
                    atb_j = atbpool.tile([128, M], BF16, tag="atb_sb")
                    nc.vector.tensor_copy(out=atb_j, in_=atb_ps)
                    atb.append(atb_j)

                # einsum2: bapT[32j+m, c] = sum_hw A[m,hw] x[c,hw]; nt chunks
                # in c-lo->c-hi order so the tail only waits for the last
                # xta half.
                featsq = fpool.tile([64, C], BF16, tag="featsq", name=f"fq{pr}")
                for nt in range(4):
                    bap_ps = ps_bap.tile(
                        [64, 512], F32, tag="bap", name=f"bap{pr}_{nt}"
                    )
                    csl = slice(512 * nt, 512 * nt + 512)
                    for j in range(2):
                        nc.tensor.matmul(
                            bap_ps[32 * j:32 * j + 32, :],
                            lhsT=ata[j],
                            rhs=xta[pr][0:68, j, csl],
                            start=True,
                            stop=False,
                            tile_position=(0, 32 * j),
                        )
                    for j in range(2):
                        nc.tensor.matmul(
                            bap_ps[32 * j:32 * j + 32, :],
                            lhsT=atb[j],
                            rhs=xtb[pr][:, j, csl],
                            start=False,
                            stop=True,
                            tile_position=(0, 32 * j),
                        )
                    if nt % 2 == 0:
                        nc.vector.tensor_copy(out=featsq[:, csl], in_=bap_ps)
                    else:
                        nc.scalar.copy(out=featsq[:, csl], in_=bap_ps)

                # store: early pairs on the idle SWDGE queue; the last pair
                # in column halves on sync (its loads are done by then)
                fap = feats.ap()[2 * pr:2 * pr + 2].rearrange(
                    "b (m c) -> (b m) c", m=M
                )
                if pr < NP - 1:
                    nc.gpsimd.dma_start(out=fap, in_=featsq)
                else:
                    nc.sync.dma_start(out=fap[:, 0:1024], in_=featsq[:, 0:1024])
                    nc.sync.dma_start(
                        out=fap[:, 1024:2048], in_=featsq[:, 1024:2048]
                    )
    nc.compile()
    return nc


def _build_phase2():
    """Per-core: featsT slice (partition-major, bf16) x WcT slice (bf16)
    -> partial [B, NCLS] (fp32)."""
    nc = _nc()
    ft = nc.dram_tensor("ft", [128, KT, B], BF16, kind="ExternalInput")
    wct = nc.dram_tensor("wct", [128, KT, NCLS], BF16, kind="ExternalInput")
    part = nc.dram_tensor("part", [B, NCLS], F32, kind="ExternalOutput")

    # (kt0, ch, engine_idx): 0 = sync, 1 = scalar.  Chunks alternate queues
    # in strict kt order so matmul progress tracks both queues' cumulative
    # bytes with no queue-jumping stalls; small chunks at the head (early
    # matmul start) and tail (short compute tail).
    wchunks = [
        (0, 4, 0), (4, 4, 1), (8, 8, 0), (16, 8, 1), (24, 8, 0),
        (32, 8, 1), (40, 8, 0), (48, 8, 1), (56, 4, 0), (60, 4, 1),
    ]

    with TileContext(nc) as tc:
        with (
            tc.tile_pool(name="fpool", bufs=1) as fpool,
            tc.tile_pool(name="wpool", bufs=10) as wpool,
            tc.tile_pool(name="opool", bufs=1) as opool,
            tc.tile_pool(name="ps_out", bufs=1, space="PSUM") as ps_out,
        ):
            engs = [nc.sync, nc.scalar]
            ft_sb = fpool.tile([128, KT, B], BF16)
            # emission order per engine == queue order; ft chunks interleave
            # with wct chunks so each arrives just before the matmuls that
            # need it, and both queues' cumulative bytes stay equal
            nc.sync.dma_start(out=ft_sb[:, 0:16, :], in_=ft.ap()[:, 0:16, :])
            nc.scalar.dma_start(out=ft_sb[:, 16:32, :], in_=ft.ap()[:, 16:32, :])
            w_sb = {}

            def load_w(kt0, ch, e):
                w_sb[kt0] = wpool.tile([128, ch, NCLS], BF16, tag="w",
                                       name=f"w{kt0}")
                engs[e].dma_start(
                    out=w_sb[kt0], in_=wct.ap()[:, kt0:kt0 + ch, :]
                )

            for kt0, ch, e in wchunks[:4]:
                load_w(kt0, ch, e)
            nc.sync.dma_start(out=ft_sb[:, 32:48, :], in_=ft.ap()[:, 32:48, :])
            nc.scalar.dma_start(out=ft_sb[:, 48:64, :], in_=ft.ap()[:, 48:64, :])
            for kt0, ch, e in wchunks[4:]:
                load_w(kt0, ch, e)

            # col-paired matmuls: even kt accumulate into PSUM partitions
            # 0:64 (col group pair 0), odd kt into 64:128, so consecutive
            # k-tiles run concurrently in the PE array.
            out_ps = ps_out.tile([128, NCLS], F32)
            for kt0, ch, e in wchunks:
                for kl in range(ch):
                    kt = kt0 + kl
                    half = kt % 2
                    nc.tensor.matmul(
                        out_ps[64 * half:64 * half + 64, :],
                        lhsT=ft_sb[:, kt, :],
                        rhs=w_sb[kt0][:, kl, :],
                        start=(kt < 2),
                        stop=(kt >= KT - 2),
                        tile_position=(0, 64 * half),
                    )
            hi_sb = opool.tile([B, NCLS], F32, tag="hi")
            nc.scalar.copy(out=hi_sb, in_=out_ps[64:128, :])
            out_sb = opool.tile([B, NCLS], F32, tag="out")
            nc.vector.tensor_tensor(
                out=out_sb, in0=out_ps[0:64, :], in1=hi_sb,
                op=mybir.AluOpType.add,
            )
            nc.sync.dma_start(out=part.ap(), in_=out_sb)
    nc.compile()
    return nc


def _install_ntff_hook():
    import types

    import trn_agent_boot.trn_boot as tb
    import concourse.bass_utils as bu

    hook = tb._ntff_profile_via_ctypes("/opt/axon/libaxon_pjrt.so")
    mod = types.ModuleType("antenv.axon_hooks")
    mod.get_axon_ntff_profile_hook = lambda: hook
    sys.modules["antenv.axon_hooks"] = mod
    bu.upload_artifacts = lambda tmpdir: "(skipped)"


def _run(nc, in_maps, label):
    core_ids = list(range(NCORES))
    if TRACE:
        _install_ntff_hook()
        res = run_bass_kernel_spmd(nc, in_maps, core_ids, trace=True)
        TRACE_INFO[label] = res.exec_time_ns
        TRACE_RES[label] = res
    else:
        res = run_bass_kernel_spmd(nc, in_maps, core_ids)
    return res.results


def kernel(x, Wa, ba, Wc, bc):
    import ml_dtypes

    bf16 = np.dtype(ml_dtypes.bfloat16)
    x3 = np.ascontiguousarray(x, dtype=np.float32).reshape(B, C, HW)
    x = x3.astype(bf16)
    xt = np.ascontiguousarray(x.transpose(0, 2, 1))  # [B, HW, C] bf16
    # wat[p, t, m] = Wa[m, p*CT + t] — matches the kernel's permuted c layout
    wat = np.ascontiguousarray(Wa.T, dtype=np.float32).astype(bf16).reshape(
        128, CT, M
    )
    ba2 = np.tile(
        np.ascontiguousarray(ba, dtype=np.float32).reshape(M, 1), (2, 1)
    )
    wct = np.ascontiguousarray(Wc.T, dtype=np.float32).astype(bf16)  # [KTOT, NCLS]

    if "p1" not in _cache:
        _cache["p1"] = _build_phase1()
    if "p2" not in _cache:
        _cache["p2"] = _build_phase2()

    in1 = [
        {
            "x": x[i * BPC: (i + 1) * BPC],
            "xt": xt[i * BPC: (i + 1) * BPC],
            "wat": wat,
            "ba2": ba2,
        }
        for i in range(NCORES)
    ]
    res1 = _run(_cache["p1"], in1, "phase1")
    feats = np.concatenate([r["feats"] for r in res1], axis=0)  # [B, KTOT] bf16

    # ft[p, t, b] = feats[b, kslice + t*128 + p] (partition-major, bf16)
    featsT = np.ascontiguousarray(feats.T)  # [KTOT, B]
    in2 = [
        {
            "ft": np.ascontiguousarray(
                featsT[i * KPC: (i + 1) * KPC].reshape(KT, 128, B).transpose(
                    1, 0, 2
                )
            ),
            "wct": np.ascontiguousarray(
                wct[i * KPC: (i + 1) * KPC].reshape(KT, 128, NCLS).transpose(
                    1, 0, 2
                )
            ),
        }
        for i in range(NCORES)
    ]
    res2 = _run(_cache["p2"], in2, "phase2")
    parts = np.stack([r["part"] for r in res2], axis=0)  # [NCORES, B, NCLS]

    logits = parts.sum(axis=0) / float(HW) + np.asarray(bc, dtype=np.float32)
    return logits.astype(np.float32)
